# revision 18
# baseline (speedup 1.0000x reference)
"""Trainium2 Bass kernel for nn_FMG_6717328851807 (dense_transformer).

Reference computation (B=8, C=512, H=W=64, K=64, MEM=512, heads=8, d=64):
    q = Wq @ x            (1x1 conv)          -> [B,h,N,d], N = H*W = 4096
    k = Ft @ Wk.T, v = Ft @ Wv.T              -> [B,h,K,d]
    attn = softmax(q k^T / sqrt(d))           -> [B,h,N,K]
    out = attn @ v                            -> [B,h,N,d]
    y = x + Wp @ out + bp

Sharding: pure data-parallel over B — one batch element per NeuronCore,
no collectives. Within a core everything is computed in "transposed"
layout (channels on partitions, spatial N on the free dim) so every
matmul runs with a 512-wide bf16 moving operand at the PE's full rate
(fp32 PSUM accumulation throughout; the fp32 residual path stays exact):

    qT[C,N]      = WqT.T @ bf16(x)      (16 MMs / 512-col chunk)
    kT[C,K]      = WkT.T @ FtT          (once)
    v[K,C]       = FtT.T @ WvT          (once, duplicated on partitions
                                         0-63 and 64-127 for pair-packing)
    scoresT[k,n] = kT_h.T @ qT_h        (heads packed in pairs; even/odd
                                         head matmuls land on disjoint
                                         64x64 quadrants of the PE array
                                         and run concurrently)
    expT         = exp(scoresT / 8)     (ScalarE, PSUM -> SBUF, bf16 out)
    sums[8,n]    = blockones.T @ expT   (PE, accumulated over head pairs)
    outT_h       = v_h.T @ expT_h       (pair-packed like scoresT)
    outT        *= 1/sums               (partition-broadcast of the DVE
                                         reciprocal via a DRAM bounce +
                                         0-step DMA read, DVE multiply)
    y            = WpT.T @ outT + bp(K=1 matmul row) + x (DVE add)

The y projection runs one chunk behind the rest of the pipeline so the
PE never waits on the softmax-normalize latency chain, and a short
warm-up matmul burst at kernel start brings the PE HAM clock to 2.4 GHz
while the weights stream in.
"""

import numpy as np

import concourse.bass as bass
import concourse.mybir as mybir
import concourse.tile as tile
from concourse import bacc
from concourse.bass_utils import run_bass_kernel_spmd

F32 = mybir.dt.float32
F32R = mybir.dt.float32r
BF16 = mybir.dt.bfloat16

B, C, N = 8, 512, 4096
HW = 64
K, MEM, H, D = 64, 512, 8, 64
NW = 512                # columns of N processed per chunk
NCH = N // NW           # 8 chunks
CCH = C // 128          # 4 chunks of channels/partitions
N_CORES = 8
WARMUP_MMS = 20


def _pbcast_src(row_ap):
    """AP reading one DRAM row, broadcast over 64 partitions."""
    ap = [[0, 64]] + [list(p) for p in row_ap.ap[1:]]
    return bass.AP(tensor=row_ap.tensor, offset=row_ap.offset, ap=ap)


def build_bass():
    nc = bacc.Bacc("TRN2", target_bir_lowering=False, debug=False)

    xb = nc.dram_tensor("xb", [C, N], F32, kind="ExternalInput")
    xbbf = nc.dram_tensor("xbbf", [C, N], BF16, kind="ExternalInput")
    ftT = nc.dram_tensor("ftT", [MEM, K], BF16, kind="ExternalInput")
    wqT = nc.dram_tensor("wqT", [C, C], BF16, kind="ExternalInput")
    wkT = nc.dram_tensor("wkT", [MEM, C], BF16, kind="ExternalInput")
    wvT = nc.dram_tensor("wvT", [MEM, C], BF16, kind="ExternalInput")
    wpT = nc.dram_tensor("wpT", [C, C], BF16, kind="ExternalInput")
    bpv = nc.dram_tensor("bpv", [1, C], BF16, kind="ExternalInput")
    onesb = nc.dram_tensor("onesb", [C, H], BF16, kind="ExternalInput")
    selb = nc.dram_tensor("selb", [H, C], F32, kind="ExternalInput")
    yb = nc.dram_tensor("yb", [C, N], F32, kind="ExternalOutput")

    with tile.TileContext(nc) as tc:
        _body(tc, xb, xbbf, ftT, wqT, wkT, wvT, wpT, bpv, onesb, selb, yb)
    nc.compile()
    return nc


def _body(tc, xb, xbbf, ftT, wqT, wkT, wvT, wpT, bpv, onesb, selb, yb):
    nc = tc.nc
    Exp = mybir.ActivationFunctionType.Exp

    with (
        tc.tile_pool(name="const", bufs=1) as const,
        tc.tile_pool(name="xin", bufs=3) as xin,
        tc.tile_pool(name="xbf", bufs=2) as xbfp,
        tc.tile_pool(name="qt", bufs=2) as qtp,
        tc.tile_pool(name="expt", bufs=2) as expp,
        tc.tile_pool(name="bcast", bufs=2) as bcp,
        tc.tile_pool(name="outt", bufs=2) as outp,
        tc.tile_pool(name="yout", bufs=2) as yop,
        tc.tile_pool(name="recip", bufs=2) as rcp,
        tc.tile_pool(name="ps_qy", bufs=2, space="PSUM") as ps_qy,
        tc.tile_pool(name="ps_s", bufs=2, space="PSUM") as ps_s,
        tc.tile_pool(name="ps_sum", bufs=1, space="PSUM") as ps_sum,
        tc.tile_pool(name="ps_o", bufs=2, space="PSUM") as ps_o,
        tc.tile_pool(name="ps_bc", bufs=1, space="PSUM") as ps_bc,
    ):
        # ---- PE warm-up: release the HAM clock gate while weights load -----
        wrm = const.tile([128, NW], BF16, tag="wrm")
        nc.vector.memset(wrm[:], 0.0)
        pw = ps_qy.tile([128, NW], F32, tag="qy")
        for _ in range(WARMUP_MMS):
            nc.tensor.matmul(pw[:], lhsT=wrm[:, :128], rhs=wrm[:],
                             start=True, stop=True)

        # ---- load constants ------------------------------------------------
        def load_rows(dram, ncols):
            tiles = []
            for j in range(CCH):
                t = const.tile([128, ncols], BF16, tag=f"{dram.name}{j}")
                nc.sync.dma_start(out=t[:], in_=dram[128 * j:128 * (j + 1), :])
                tiles.append(t)
            return tiles

        wq_sb = load_rows(wqT, C)
        wp_sb = load_rows(wpT, C)
        wk_sb = load_rows(wkT, C)
        wv_sb = load_rows(wvT, C)
        ftT_sb = load_rows(ftT, K)
        ones_sb = load_rows(onesb, H)
        bp_sb = const.tile([1, C], BF16, tag="bp")
        nc.sync.dma_start(out=bp_sb[:], in_=bpv[:, :])
        onerow = const.tile([1, NW], BF16, tag="onerow")
        nc.vector.memset(onerow[:], 1.0)
        sel_sb = const.tile([8, C], F32R, tag="sel")
        nc.sync.dma_start(out=sel_sb[:], in_=selb[:, :].bitcast(F32R))

        # ---- kT = Wk @ Ft^T  [C, K] ----------------------------------------
        kT_sb = []
        for cj in range(CCH):
            pk = ps_s.tile([128, NW], F32, tag="ps")
            for mk in range(CCH):
                nc.tensor.matmul(
                    pk[:, :K],
                    lhsT=wk_sb[mk][:, 128 * cj:128 * (cj + 1)],
                    rhs=ftT_sb[mk][:],
                    start=(mk == 0),
                    stop=(mk == CCH - 1),
                )
            t = const.tile([128, K], BF16, tag=f"kT{cj}")
            nc.scalar.copy(t[:], pk[:, :K])
            kT_sb.append(t)

        # ---- v = Ft @ Wv^T  [K, C], duplicated on both partition halves ----
        v_dup = const.tile([128, C], BF16, tag="vdup")
        pv = ps_o.tile([128, NW], F32, tag="po")
        for mk in range(CCH):
            nc.tensor.matmul(
                pv[0:64, :],
                lhsT=ftT_sb[mk][:],
                rhs=wv_sb[mk][:],
                start=(mk == 0),
                stop=(mk == CCH - 1),
            )
        nc.scalar.copy(v_dup[0:64, :], pv[0:64, :])
        nc.sync.dma_start(out=v_dup[64:128, :], in_=v_dup[0:64, :])

        # ---- pipelined y-projection stage (one chunk behind) ---------------
        def y_stage(i, outT_sb, x_big):
            csl = bass.ts(i, NW)
            yo = yop.tile([128, CCH, NW], F32, tag="yo")
            for m in range(CCH):
                py = ps_qy.tile([128, NW], F32, tag="qy")
                for k2 in range(CCH):
                    nc.tensor.matmul(
                        py[:],
                        lhsT=wp_sb[k2][:, 128 * m:128 * (m + 1)],
                        rhs=outT_sb[k2][:],
                        start=(k2 == 0),
                        stop=False,
                    )
                nc.tensor.matmul(
                    py[:],
                    lhsT=bp_sb[:, 128 * m:128 * (m + 1)],
                    rhs=onerow[:],
                    start=False,
                    stop=True,
                )
                nc.vector.tensor_add(yo[:, m, :], py[:], x_big[:, m, :])
            nc.sync.dma_start(
                out=yb[:, csl].rearrange("(m p) c -> p m c", p=128), in_=yo[:]
            )

        # ---- main loop over spatial chunks ---------------------------------
        prev = None
        for i in range(NCH):
            csl = bass.ts(i, NW)

            x_big = xin.tile([128, CCH, NW], F32, tag="x")
            nc.sync.dma_start(
                out=x_big[:],
                in_=xb[:, csl].rearrange("(j p) c -> p j c", p=128),
            )
            x_bf = xbfp.tile([128, CCH, NW], BF16, tag="xbf")
            nc.sync.dma_start(
                out=x_bf[:],
                in_=xbbf[:, csl].rearrange("(j p) c -> p j c", p=128),
            )

            # qT chunk [C, NW]
            qT_sb = []
            for m in range(CCH):
                pq = ps_qy.tile([128, NW], F32, tag="qy")
                for k2 in range(CCH):
                    nc.tensor.matmul(
                        pq[:],
                        lhsT=wq_sb[k2][:, 128 * m:128 * (m + 1)],
                        rhs=x_bf[:, k2, :],
                        start=(k2 == 0),
                        stop=(k2 == CCH - 1),
                    )
                t = qtp.tile([128, NW], BF16, tag=f"q{m}")
                nc.scalar.copy(t[:], pq[:])
                qT_sb.append(t)

            # scoresT + exp, head pair j = heads (2j, 2j+1)
            expT_sb = []
            for j in range(CCH):
                ps = ps_s.tile([128, NW], F32, tag="ps")
                for half in range(2):
                    lo, hi = 64 * half, 64 * half + 64
                    nc.tensor.matmul(
                        ps[lo:hi, :],
                        lhsT=kT_sb[j][lo:hi, :],
                        rhs=qT_sb[j][lo:hi, :],
                        start=True,
                        stop=True,
                    )
                t = expp.tile([128, NW], BF16, tag=f"e{j}")
                nc.scalar.activation(t[:], ps[:], Exp, bias=0.0, scale=0.125)
                expT_sb.append(t)

            # per-head softmax denominators [H, NW], reciprocal in 4 slices
            psum = ps_sum.tile([8, NW], F32, tag="psum")
            for j in range(CCH):
                nc.tensor.matmul(
                    psum[:],
                    lhsT=ones_sb[j][:, :8],
                    rhs=expT_sb[j][:],
                    start=(j == 0),
                    stop=(j == CCH - 1),
                )
            recip = rcp.tile([8, NW], F32R, tag="recip")
            with nc.allow_low_precision(reason="f32r softmax reciprocal"):
                nc.vector.reciprocal(recip[:], psum[:])

            # y projection for the previous chunk (overlaps this chunk's
            # softmax-normalize latency chain)
            if prev is not None:
                y_stage(i - 1, *prev)

            # broadcast 1/sum rows across the partition halves on the PE:
            # bc[p, n] = sel[head(p), 128j + p].T @ recip -> PSUM, then ACT
            bc_sb = []
            for j in range(CCH):
                pb = ps_bc.tile([128, NW], F32, tag="pb")
                nc.tensor.matmul(
                    pb[:],
                    lhsT=sel_sb[:, 128 * j:128 * (j + 1)],
                    rhs=recip[:],
                    start=True,
                    stop=True,
                )
                t = bcp.tile([128, NW], F32, tag=f"b{j}")
                nc.scalar.copy(t[:], pb[:])
                bc_sb.append(t)

            # outT = v^T @ expT per head (pair-packed), then normalize
            outT_sb = []
            for j in range(CCH):
                po = ps_o.tile([128, NW], F32, tag="po")
                for half in range(2):
                    lo, hi = 64 * half, 64 * half + 64
                    nc.tensor.matmul(
                        po[lo:hi, :],
                        lhsT=v_dup[lo:hi, 128 * j + 64 * half:
                                   128 * j + 64 * half + 64],
                        rhs=expT_sb[j][lo:hi, :],
                        start=True,
                        stop=True,
                    )
                t = outp.tile([128, NW], BF16, tag=f"o{j}")
                nc.vector.tensor_mul(t[:], po[:], bc_sb[j][:])
                outT_sb.append(t)

            prev = (outT_sb, x_big)

        y_stage(NCH - 1, *prev)


_NC_CACHE = None
LAST_RESULTS = None


def kernel(x, Ft, Wq, Wk, Wv, Wp, bp):
    global _NC_CACHE, LAST_RESULTS
    import ml_dtypes

    bf16 = ml_dtypes.bfloat16
    x = np.ascontiguousarray(np.asarray(x, dtype=np.float32))
    Ft = np.asarray(Ft, dtype=np.float32)

    wqT = np.ascontiguousarray(np.asarray(Wq, dtype=np.float32).T).astype(bf16)
    wkT = np.ascontiguousarray(np.asarray(Wk, dtype=np.float32).T).astype(bf16)
    wvT = np.ascontiguousarray(np.asarray(Wv, dtype=np.float32).T).astype(bf16)
    wpT = np.ascontiguousarray(np.asarray(Wp, dtype=np.float32).T).astype(bf16)
    bpv = np.asarray(bp, dtype=np.float32).reshape(1, C).astype(bf16)
    onesb = np.zeros((C, H), dtype=np.float32)
    onesb[np.arange(C), np.arange(C) // D] = 1.0
    selb = np.ascontiguousarray(onesb.T)
    onesb = onesb.astype(bf16)
    ftT = np.ascontiguousarray(Ft.transpose(0, 2, 1)).astype(bf16)
    xr = x.reshape(B, C, N)
    xrbf = xr.astype(bf16)

    if _NC_CACHE is None:
        _NC_CACHE = build_bass()
    nc = _NC_CACHE

    in_maps = [
        {
            "xb": xr[b],
            "xbbf": xrbf[b],
            "ftT": ftT[b],
            "wqT": wqT,
            "wkT": wkT,
            "wvT": wvT,
            "wpT": wpT,
            "bpv": bpv,
            "onesb": onesb,
            "selb": selb,
        }
        for b in range(B)
    ]
    res = run_bass_kernel_spmd(nc, in_maps, core_ids=list(range(N_CORES)))
    LAST_RESULTS = res
    y = np.stack([res.results[b]["yb"] for b in range(B)])
    return y.reshape(B, C, HW, HW)
